# revision 4
# baseline (speedup 1.0000x reference)
"""Trainium2 Bass kernel for nn_MultiHeadAttention_55654186221650.

Dual-softmax linear attention (GDN) with persistent memory tokens.

Sharding: 8 cores = (batch 4) x (head-halves 2). Each core handles one batch
element and 8 of the 16 heads end-to-end (QKV projections, dual softmax, KV
contraction, WO partial product). The two cores of a batch pair produce
additive partial outputs through WO; the host sums them. No collectives.

Per-core math (natural token-on-partition layout):
  k = K[b] @ WKc.T            (4096 x 512), + 512 persistent MK rows
  expk = exp(k * dk^0.25)
  den[d]    = sum_m expk[m, d]                      (PSUM matmul w/ ones col)
  kvT[e, d] = sum_m v[m, e] * expk[m, d]            (PSUM accumulate, 36 chunks)
  G[d, o]   = sum_e (kvT[e, d] / den[d]) * WO.T[e, o]   (tiny, blockdiag/head)
  beta = rowwise-softmax_per-head(exp(q))
  y_partial[n, o] = sum_d beta[n, d] * G[d, o]      (betaT via PE transpose)

Matmuls run in float32r (single-pass reduced-precision fp32, ~1e-3 rel err)
with fp32 PSUM accumulation; free dims kept >= 256 for full PE rate.
"""

import numpy as np
from contextlib import ExitStack

import concourse.bass as bass
import concourse.tile as tile
from concourse import mybir, bacc, bass_utils
from concourse.masks import make_identity

# Problem constants (hardcoded per contract)
H = 16
DK = 64
NP = 512            # persistent tokens
D = 1024            # d_model
BS = 4
NT = 4096           # sequence length
HD = 512            # head dims per core (8 heads x 64)
NCH = NT // 128     # 32 token chunks
PCH = NP // 128     # 4 persistent chunks
DCH = D // 128      # 8 contraction chunks
SCALE = float(DK ** 0.25)

F32 = mybir.dt.float32
F32R = mybir.dt.float32r

_PROG_CACHE = {}


def build_program(reps: int = 1):
    if reps in _PROG_CACHE:
        return _PROG_CACHE[reps]
    nc = bacc.Bacc("TRN2", target_bir_lowering=False, debug=False)
    KT = nc.dram_tensor("KT", [D, NT], F32R, kind="ExternalInput").ap()
    VT = nc.dram_tensor("VT", [D, NT], F32R, kind="ExternalInput").ap()
    QT = nc.dram_tensor("QT", [D, NT], F32R, kind="ExternalInput").ap()
    WKt = nc.dram_tensor("WKt", [D, HD], F32R, kind="ExternalInput").ap()
    WVt = nc.dram_tensor("WVt", [D, HD], F32R, kind="ExternalInput").ap()
    WQt = nc.dram_tensor("WQt", [D, HD], F32R, kind="ExternalInput").ap()
    E = nc.dram_tensor("E", [HD, D], F32R, kind="ExternalInput").ap()
    MKc = nc.dram_tensor("MKc", [NP, HD], F32, kind="ExternalInput").ap()
    MVc = nc.dram_tensor("MVc", [NP, HD], F32R, kind="ExternalInput").ap()
    Y = nc.dram_tensor("Y", [NT, D], F32, kind="ExternalOutput").ap()

    KTr = KT.rearrange("(c p) t -> p c t", p=128)
    VTr = VT.rearrange("(c p) t -> p c t", p=128)
    QTr = QT.rearrange("(c p) t -> p c t", p=128)
    WKr = WKt.rearrange("(c p) n -> p c n", p=128)
    WVr = WVt.rearrange("(c p) n -> p c n", p=128)
    WQr = WQt.rearrange("(c p) n -> p c n", p=128)
    Er = E.rearrange("(c p) n -> p c n", p=128)

    with tile.TileContext(nc) as tc:
        with ExitStack() as ctx:
            const = ctx.enter_context(tc.tile_pool(name="const", bufs=1))
            gbuf = ctx.enter_context(tc.tile_pool(name="gbuf", bufs=2))
            lk = ctx.enter_context(tc.tile_pool(name="lk", bufs=3))
            lv = ctx.enter_context(tc.tile_pool(name="lv", bufs=3))
            lq = ctx.enter_context(tc.tile_pool(name="lq", bufs=3))
            se = ctx.enter_context(tc.tile_pool(name="se", bufs=2))
            sv = ctx.enter_context(tc.tile_pool(name="sv", bufs=2))
            sq = ctx.enter_context(tc.tile_pool(name="sq", bufs=2))
            sy = ctx.enter_context(tc.tile_pool(name="sy", bufs=2))

            # Resident weights
            wk_t = const.tile([128, DCH, HD], F32R)
            wv_t = const.tile([128, DCH, HD], F32R)
            wq_t = const.tile([128, DCH, HD], F32R)
            for c in range(DCH):
                nc.sync.dma_start(out=wk_t[:, c, :], in_=WKr[:, c, :])
                nc.sync.dma_start(out=wv_t[:, c, :], in_=WVr[:, c, :])
                nc.sync.dma_start(out=wq_t[:, c, :], in_=WQr[:, c, :])
            e_t = const.tile([128, 4, D], F32R)
            for c in range(4):
                nc.sync.dma_start(out=e_t[:, c, :], in_=Er[:, c, :])
            ident = const.tile([128, 128], F32)
            make_identity(nc, ident)
            # f32r constants: DVE memset can't write f32r, so memset f32 + copy
            ones_f = const.tile([128, 2], F32)
            nc.vector.memset(ones_f, 1.0)
            ones = const.tile([128, 2], F32R)
            nc.vector.tensor_copy(out=ones, in_=ones_f)
            zeros_f = const.tile([128, 4, 128], F32)
            nc.vector.memset(zeros_f, 0.0)

            for rep in range(reps):
                g_sb = gbuf.tile([128, 4, D], F32R, tag="g_sb")
                with tc.tile_pool(name="psacc", bufs=1, space="PSUM") as psacc:
                    kv_ps = psacc.tile([128, 4, HD], F32)
                    den_ps = psacc.tile([128, 8], F32)

                    # ---- Phase KV: 32 projected chunks + 4 persistent chunks
                    with tc.tile_pool(name="pk", bufs=2, space="PSUM") as pk, \
                         tc.tile_pool(name="pv", bufs=1, space="PSUM") as pv:
                        for i in range(NCH + PCH):
                            first = i == 0
                            last = i == NCH + PCH - 1
                            if i < NCH:
                                tsl = bass.ts(i, 128)
                                kt_t = lk.tile([128, DCH, 128], F32R, tag="kt")
                                vt_t = lv.tile([128, DCH, 128], F32R, tag="vt")
                                nc.sync.dma_start(out=kt_t, in_=KTr[:, :, tsl])
                                nc.sync.dma_start(out=vt_t, in_=VTr[:, :, tsl])
                                # v projection -> v_sb (f32r)
                                psv = pv.tile([128, HD], F32, tag="psv")
                                for c in range(DCH):
                                    nc.tensor.matmul(psv, vt_t[:, c, :], wv_t[:, c, :],
                                                     start=(c == 0), stop=(c == DCH - 1))
                                v_sb = sv.tile([128, HD], F32R, tag="v_sb")
                                nc.vector.tensor_copy(out=v_sb, in_=psv)
                                # k projection -> expk (f32r)
                                psk = pk.tile([128, HD], F32, tag="psk")
                                for c in range(DCH):
                                    nc.tensor.matmul(psk, kt_t[:, c, :], wk_t[:, c, :],
                                                     start=(c == 0), stop=(c == DCH - 1))
                                expk = se.tile([128, HD], F32R, tag="expk")
                                nc.scalar.activation(out=expk, in_=psk,
                                                     func=mybir.ActivationFunctionType.Exp,
                                                     scale=SCALE)
                            else:
                                pc = i - NCH
                                psl = bass.ts(pc, 128)
                                mk_t = lk.tile([128, HD], F32, tag="mk")
                                nc.sync.dma_start(out=mk_t, in_=MKc[psl, :])
                                v_sb = sv.tile([128, HD], F32R, tag="v_sb")
                                nc.sync.dma_start(out=v_sb, in_=MVc[psl, :])
                                expk = se.tile([128, HD], F32R, tag="expk")
                                nc.scalar.activation(out=expk, in_=mk_t,
                                                     func=mybir.ActivationFunctionType.Exp,
                                                     scale=SCALE)
                            # den += expk_pair.T @ ones ; kvT += v_pair.T @ expk
                            for hp in range(4):
                                csl = bass.ts(hp, 128)
                                nc.tensor.matmul(den_ps[:, 2 * hp:2 * hp + 2],
                                                 expk[:, csl], ones,
                                                 start=(first and hp == 0), stop=last,
                                                 skip_group_check=True)
                            for hp in range(4):
                                csl = bass.ts(hp, 128)
                                nc.tensor.matmul(kv_ps[:, hp, :], v_sb[:, csl], expk,
                                                 start=first, stop=last,
                                                 skip_group_check=True)

                    # ---- Phase G: G[d, o] = (kvT[e, d]/den[d]).T @ E
                    with tc.tile_pool(name="pg", bufs=2, space="PSUM") as pg:
                        kvsb = gbuf.tile([128, 4, 128], F32R, tag="kvsb")
                        nc.vector.tensor_copy(out=kvsb, in_=zeros_f)
                        den_r = gbuf.tile([128, 8], F32, tag="den_r")
                        nc.vector.reciprocal(out=den_r, in_=den_ps)
                        for hp in range(4):
                            for h2 in range(2):
                                rsl = slice(h2 * 64, (h2 + 1) * 64)
                                nc.vector.tensor_copy(
                                    out=kvsb[rsl, hp, h2 * 64:(h2 + 1) * 64],
                                    in_=kv_ps[rsl, hp, hp * 128 + h2 * 64: hp * 128 + (h2 + 1) * 64])
                        for hp in range(4):
                            for oc in range(2):
                                psg = pg.tile([128, 512], F32, tag="psg")
                                nc.tensor.matmul(psg, kvsb[:, hp, :],
                                                 e_t[:, hp, bass.ts(oc, 512)],
                                                 start=True, stop=True)
                                nc.vector.tensor_scalar_mul(
                                    g_sb[:, hp, bass.ts(oc, 512)], psg,
                                    den_r[:, 2 * hp:2 * hp + 1])

                # ---- Phase Q: beta softmax + y = betaT.T @ G
                with tc.tile_pool(name="pq", bufs=2, space="PSUM") as pq, \
                     tc.tile_pool(name="ptr", bufs=2, space="PSUM") as ptr, \
                     tc.tile_pool(name="py", bufs=4, space="PSUM") as py:
                    for i in range(NCH):
                        tsl = bass.ts(i, 128)
                        qt_t = lq.tile([128, DCH, 128], F32R, tag="qt")
                        nc.sync.dma_start(out=qt_t, in_=QTr[:, :, tsl])
                        psq = pq.tile([128, HD], F32, tag="psq")
                        for c in range(DCH):
                            nc.tensor.matmul(psq, qt_t[:, c, :], wq_t[:, c, :],
                                             start=(c == 0), stop=(c == DCH - 1))
                        expq = sq.tile([128, HD], F32, tag="expq")
                        nc.scalar.activation(out=expq, in_=psq,
                                             func=mybir.ActivationFunctionType.Exp)
                        expq3 = expq.rearrange("p (h e) -> p h e", h=8)
                        s_t = sq.tile([128, 8], F32, tag="s_t")
                        nc.vector.reduce_sum(out=s_t, in_=expq3,
                                             axis=mybir.AxisListType.X)
                        r_t = sq.tile([128, 8], F32, tag="r_t")
                        nc.vector.reciprocal(out=r_t, in_=s_t)
                        beta = sq.tile([128, HD], F32, tag="beta")
                        beta3 = beta.rearrange("p (h e) -> p h e", h=8)
                        nc.vector.tensor_mul(beta3, expq3,
                                             r_t.unsqueeze(2).broadcast_to((128, 8, 64)))
                        pst = ptr.tile([128, 512], F32, tag="pst")
                        for hp in range(4):
                            nc.tensor.transpose(pst[:, bass.ts(hp, 128)],
                                                beta[:, bass.ts(hp, 128)], ident)
                        btr = sq.tile([128, 4, 128], F32R, tag="btr")
                        for hp in range(4):
                            nc.vector.tensor_copy(out=btr[:, hp, :],
                                                  in_=pst[:, bass.ts(hp, 128)])
                        y_sb = sy.tile([128, 2, 512], F32, tag="y_sb")
                        for oc in range(2):
                            psy = py.tile([128, 512], F32, tag="psy")
                            for hp in range(4):
                                nc.tensor.matmul(psy, btr[:, hp, :],
                                                 g_sb[:, hp, bass.ts(oc, 512)],
                                                 start=(hp == 0), stop=(hp == 3))
                            nc.scalar.copy(out=y_sb[:, oc, :], in_=psy)
                        nc.sync.dma_start(out=Y[tsl, :],
                                          in_=y_sb.rearrange("p a b -> p (a b)"))
    nc.compile()
    _PROG_CACHE[reps] = nc
    return nc


def make_in_maps(Q, K, V, WQ, WK, WV, WO, MK, MV):
    in_maps = []
    for b in range(BS):
        for hg in range(2):
            sl = slice(hg * HD, (hg + 1) * HD)
            hsl = slice(hg * 8, (hg + 1) * 8)
            in_maps.append({
                "KT": np.ascontiguousarray(K[b].T),
                "VT": np.ascontiguousarray(V[b].T),
                "QT": np.ascontiguousarray(Q[b].T),
                "WKt": np.ascontiguousarray(WK[sl, :].T),
                "WVt": np.ascontiguousarray(WV[sl, :].T),
                "WQt": np.ascontiguousarray(WQ[sl, :].T),
                "E": np.ascontiguousarray(WO[:, sl].T),
                "MKc": np.ascontiguousarray(
                    MK[0, hsl].transpose(1, 0, 2).reshape(NP, HD)),
                "MVc": np.ascontiguousarray(
                    MV[0, hsl].transpose(1, 0, 2).reshape(NP, HD)),
            })
    return in_maps


def run_spmd(nc, in_maps):
    return bass_utils.run_bass_kernel_spmd(nc, in_maps, core_ids=list(range(8)))


def kernel(Q, K, V, WQ, WK, WV, WO, MK, MV):
    Q = np.asarray(Q, dtype=np.float32)
    K = np.asarray(K, dtype=np.float32)
    V = np.asarray(V, dtype=np.float32)
    nc = build_program(reps=1)
    in_maps = make_in_maps(Q, K, V, np.asarray(WQ, np.float32),
                           np.asarray(WK, np.float32), np.asarray(WV, np.float32),
                           np.asarray(WO, np.float32), np.asarray(MK, np.float32),
                           np.asarray(MV, np.float32))
    res = run_spmd(nc, in_maps)
    out = np.zeros((BS, NT, D), dtype=np.float32)
    for b in range(BS):
        out[b] = res.results[2 * b]["Y"] + res.results[2 * b + 1]["Y"]
    return out
